# revision 1
# baseline (speedup 1.0000x reference)
"""5G LDPC BG1 encoder (k=8000, n=16000, r=0.5, Z=384) on 8 Trainium2 cores.

Strategy: pure data parallelism over the batch (2048 -> 8 cores x 256 rows,
2 partition-tiles of 128 each). Bits are kept as bf16 0.0/1.0 on the free
axis; GF(2) addition is bitwise XOR on the raw bit patterns (0x3F80 ^ 0x3F80
= 0x0000), so no mod-2 is ever needed. Circulant lifted blocks (Z=384) are
handled by keeping a duplicated "halo" copy of every 384-col block so a
cyclic shift is a single contiguous slice -> one elementwise op per
base-graph entry.  Rate matching only emits extension parity bits pb[0:7232]
(19 of 42 blocks), so the other 23 blocks are never computed.  The output
interleaver (out[:, 4j+i] = c_short[i*4000+j]) is fused with the bf16->f32
up-conversion as stride-4 copies on the Activation engine, emitted per
4000-column output chunk so chunk DMAs overlap compute.  XOR work is split
DVE/GpSimd to balance engine busy time.
"""
import numpy as np
from contextlib import ExitStack

Z = 384
KB = 22
MB = 46
K = 8000
N = 16000
K_LDPC = KB * Z          # 8448
M_A = 4 * Z              # 1536
NBPS = 4
NQ = N // NBPS           # 4000
PB_BLOCKS = 19           # only pb[0:7232] survives rate matching

B_TOTAL = 2048
N_CORES = 8
B_CORE = B_TOTAL // N_CORES   # 256
P = 128
TILES = B_CORE // P           # 2
NCHUNK = 4                    # output column chunks of 4000

_CACHE = {}


def _base_entries(rows, cols):
    """Recover (base_row, base_col, shift) triplets from lifted index lists."""
    rows = np.asarray(rows, np.int64)
    cols = np.asarray(cols, np.int64)
    m = (rows % Z) == 0
    br = (rows[m] // Z).astype(int)
    bc = (cols[m] // Z).astype(int)
    sh = (cols[m] % Z).astype(int)
    return list(zip(br.tolist(), bc.tolist(), sh.tolist()))


def _group(entries, n_blocks, drop_bc=()):
    g = [[] for _ in range(n_blocks)]
    for br, bc, s in entries:
        if bc in drop_bc or br >= n_blocks:
            continue
        g[br].append((bc, s))
    return g


def _ilv_copies(chunk):
    """Interleaver copy specs for output chunk (cols [chunk*4000, +4000)):
    (tile, blk0, off, nblk, ln, dst_start_within_chunk).

    c_short = u_bits[768:8000] ++ pa[0:1536] ++ pb[0:7232], and
    out[:, 4j+i] = c_short[i*4000 + j]; chunk c covers j in [c*1000,(c+1)*1000).
    """
    spans = ([("u", b, 0, Z) for b in range(2, 20)] + [("u", 20, 0, 320)]
             + [("pa", b, 0, Z) for b in range(4)]
             + [("pb", b, 0, Z) for b in range(18)] + [("pb", 18, 0, 320)])
    jlo, jhi = chunk * (NQ // NCHUNK), (chunk + 1) * (NQ // NCHUNK)
    out = []
    for i in range(NBPS):
        # phase i reads c_short[i*NQ + j] for j in [jlo, jhi) of this chunk
        glo, ghi = i * NQ + jlo, i * NQ + jhi
        g = 0
        pieces = []
        for tname, blk, off, ln in spans:
            a, b = max(g, glo), min(g + ln, ghi)
            if a < b:
                pieces.append((tname, blk, off + a - g, b - a,
                               4 * (a - glo) + i))
            g += ln
        merged = []
        for pc in pieces:
            tname, blk, off, ln, ds = pc
            if merged and off == 0 and ln == Z:
                mt, mb_, mo, mn, ml, mds = merged[-1]
                if mt == tname and mo == 0 and ml == Z and mb_ + mn == blk:
                    merged[-1] = (mt, mb_, mo, mn + 1, ml, mds)
                    continue
            merged.append((tname, blk, off, 1, ln, ds))
        out.extend(merged)
    return out


def _build_program(gA, gC1, gC2):
    import concourse.tile as tile
    from concourse import bacc, mybir
    from concourse.alu_op_type import AluOpType

    f32 = mybir.dt.float32
    u16 = mybir.dt.uint16
    bf16 = mybir.dt.bfloat16
    XOR = AluOpType.bitwise_xor

    nc = bacc.Bacc("TRN2", target_bir_lowering=False, debug=False)
    u_dram = nc.dram_tensor("u", [B_CORE, K], f32, kind="ExternalInput").ap()
    o_dram = nc.dram_tensor("out", [B_CORE, N], f32, kind="ExternalOutput").ap()

    with tile.TileContext(nc) as tc, ExitStack() as ctx:
        pin = ctx.enter_context(tc.tile_pool(name="pin", bufs=2))
        pw2 = ctx.enter_context(tc.tile_pool(name="pw2", bufs=2))
        pw1 = ctx.enter_context(tc.tile_pool(name="pw1", bufs=1))
        pout = ctx.enter_context(tc.tile_pool(name="pout", bufs=1))

        for t in range(TILES):
            r0 = t * P
            # ---- DMA in (block-aligned chunks) + convert to bf16 u_dup ----
            tf0 = pin.tile([P, 10, Z], f32, tag="tf")
            nc.sync.dma_start(tf0[:], u_dram[r0:r0 + P, 0:3840])
            tf1 = pin.tile([P, 10, Z], f32, tag="tf")
            nc.sync.dma_start(tf1[:], u_dram[r0:r0 + P, 3840:7680])
            tf2 = pin.tile([P, 320], f32, tag="tf2")
            nc.sync.dma_start(tf2[:], u_dram[r0:r0 + P, 7680:8000])

            # u_dup[p, bc, 0:384] = block bc ; [p, bc, 384:768] = same (halo)
            u_dup = pw2.tile([P, KB, 2 * Z], u16, tag="u_dup")
            nc.scalar.copy(u_dup[:, 0:10, 0:Z].bitcast(bf16), tf0[:])
            nc.scalar.copy(u_dup[:, 10:20, 0:Z].bitcast(bf16), tf1[:])
            nc.scalar.copy(u_dup[:, 20, 0:320].bitcast(bf16), tf2[:])
            nc.gpsimd.memset(u_dup[:, 20, 320:Z], 0)
            nc.gpsimd.memset(u_dup[:, 20, Z + 320:2 * Z], 0)
            nc.vector.tensor_copy(u_dup[:, 0:10, Z:2 * Z], u_dup[:, 0:10, 0:Z])
            nc.vector.tensor_copy(u_dup[:, 10:20, Z:2 * Z], u_dup[:, 10:20, 0:Z])
            nc.gpsimd.tensor_copy(u_dup[:, 20, Z:Z + 320], u_dup[:, 20, 0:320])

            def usrc(bc, s):
                return u_dup[:, bc, s:s + Z]

            def accumulate(eng, dst, srcs):
                """dst <- XOR of srcs (first pair direct, rest in place)."""
                if len(srcs) == 1:
                    nc.vector.tensor_copy(dst, srcs[0])
                    return
                eng.tensor_tensor(dst, srcs[0], srcs[1], op=XOR)
                for sp in srcs[2:]:
                    eng.tensor_tensor(dst, dst, sp, op=XOR)

            # ---- au = A @ u ----
            au = pw1.tile([P, 4, Z], u16, tag="au")
            for br in range(4):
                accumulate(nc.vector, au[:, br, :],
                           [usrc(bc, s) for bc, s in gA[br]])

            # ---- pa = B_inv @ au = cumulative XOR chain ----
            pa = pw1.tile([P, 4, 2 * Z], u16, tag="pa")
            nc.vector.tensor_copy(pa[:, 0, 0:Z], au[:, 0, :])
            for i in range(1, 4):
                nc.vector.tensor_tensor(pa[:, i, 0:Z], pa[:, i - 1, 0:Z],
                                        au[:, i, :], op=XOR)
            nc.gpsimd.tensor_copy(pa[:, :, Z:2 * Z], pa[:, :, 0:Z])

            def pasrc(bc, s):
                return pa[:, bc, s:s + Z]

            # ---- pb = C1 @ u + C2 @ pa (only the 19 surviving blocks) ----
            # Bitwise XOR is DVE-only on TRN2 (HW verifier rejects Pool).
            pb = pw1.tile([P, PB_BLOCKS, Z], u16, tag="pb")

            def pb_block(lr):
                srcs = [usrc(bc, s) for bc, s in gC1[lr]]
                srcs += [pasrc(bc, s) for bc, s in gC2[lr]]
                accumulate(nc.vector, pb[:, lr, :], srcs)

            # ---- interleave + bf16->f32 per output chunk, chunk DMA out ----
            # Early (u/pa-sourced, phases i=0,1) copies go to GpSimd so they
            # run during the DVE XOR burst; pb-sourced (i=2,3) go to ACT and
            # are emitted as soon as the pb blocks a chunk needs are done.
            tiles = {"u": u_dup, "pa": pa, "pb": pb}
            cw = N // NCHUNK

            def emit_ilv(of, c, want_pb):
                for tname, blk0, off, nblk, ln, ds in _ilv_copies(c):
                    if (tname == "pb") != want_pb:
                        continue
                    src_t = tiles[tname]
                    if nblk > 1:
                        dst = of[:, ds:ds + 4 * (Z * nblk - 1) + 1:4]
                        dst = dst.rearrange("p (a b) -> p a b", b=Z)
                        src = src_t[:, blk0:blk0 + nblk, 0:Z]
                    else:
                        dst = of[:, ds:ds + 4 * (ln - 1) + 1:4]
                        src = src_t[:, blk0, off:off + ln]
                    if want_pb:
                        nc.scalar.copy(dst, src.bitcast(bf16))
                    else:
                        nc.gpsimd.tensor_copy(dst, src.bitcast(bf16))

            # pb blocks needed per chunk (max block index + 1)
            need = []
            for c in range(NCHUNK):
                mx = 0
                for tname, blk0, off, nblk, ln, ds in _ilv_copies(c):
                    if tname == "pb":
                        mx = max(mx, blk0 + nblk)
                need.append(mx)

            done = 0
            for c in range(NCHUNK):
                of = pout.tile([P, cw], f32, tag=f"of{c % 2}")
                emit_ilv(of, c, want_pb=False)
                for lr in range(done, need[c]):
                    pb_block(lr)
                done = max(done, need[c])
                emit_ilv(of, c, want_pb=True)
                nc.sync.dma_start(o_dram[r0:r0 + P, c * cw:(c + 1) * cw],
                                  of[:])
            for lr in range(done, PB_BLOCKS):
                pb_block(lr)

    return nc


def _get_program(a_rows, a_cols, bi_rows, bi_cols, c1_rows, c1_cols,
                 c2_rows, c2_cols):
    if "prog" in _CACHE:
        return _CACHE["prog"]
    entB = _base_entries(bi_rows, bi_cols)
    assert sorted(entB) == [(i, j, 0) for i in range(4) for j in range(i + 1)]
    gA = _group(_base_entries(a_rows, a_cols), 4, drop_bc=(21,))
    gC1 = _group(_base_entries(c1_rows, c1_cols), PB_BLOCKS, drop_bc=(21,))
    gC2 = _group(_base_entries(c2_rows, c2_cols), PB_BLOCKS)
    nc = _build_program(gA, gC1, gC2)
    nc.compile()
    _CACHE["prog"] = nc
    return nc


def kernel(u, a_rows, a_cols, bi_rows, bi_cols, c1_rows, c1_cols,
           c2_rows, c2_cols, out_int, **_ignored):
    from concourse.bass_utils import run_bass_kernel_spmd

    u = np.ascontiguousarray(np.asarray(u, np.float32))
    assert u.shape == (B_TOTAL, K)
    oi = np.asarray(out_int)
    expect = np.arange(N, dtype=oi.dtype).reshape(NBPS, NQ).T.ravel()
    assert np.array_equal(oi, expect), "unexpected output interleaver"

    nc = _get_program(a_rows, a_cols, bi_rows, bi_cols,
                      c1_rows, c1_cols, c2_rows, c2_cols)
    in_maps = [{"u": u[i * B_CORE:(i + 1) * B_CORE]} for i in range(N_CORES)]
    res = run_bass_kernel_spmd(nc, in_maps, core_ids=list(range(N_CORES)))
    return np.concatenate([res.results[i]["out"] for i in range(N_CORES)], axis=0)



# revision 3
# speedup vs baseline: 2.2880x; 2.2880x over previous
"""5G LDPC BG1 encoder (k=8000, n=16000, r=0.5, Z=384) on 8 Trainium2 cores.

Strategy: data parallelism over the batch (2048 -> 8 cores x 256 rows) with
4-way nibble packing: 4 batch rows share one uint16 SBUF lane (row t*64+p ->
nibble t of partition p), so every engine op processes 4 codewords at once
and DMA moves 4x fewer bytes.  GF(2) addition is nibble-wise bitwise XOR
(DVE-only on TRN2); nibbles stay in {0,1} so the host recovers bits with a
shift-and-mask.  Circulant shifts use a halo copy of every 384-col block,
loaded by a second DMA pass straight from DRAM.  Independent XOR-chain steps
from two different rows are fused into one [P,2,384] DVE instruction via
hand-built access patterns (arbitrary stride between the two rows), halving
DVE instruction count.  The rate-matching interleaver (out[:,4j+i] =
c_short[i*4000+j]) runs as stride-4 packed copies split between Activation
and GpSimd: u/pa-sourced output phases are emitted early, pb-sourced spans
per chunk as parity rows complete.  Host work is layout-only: pack nibbles
in, shift-and-mask out.
"""
import numpy as np
from contextlib import ExitStack

Z = 384
KB = 22
K = 8000
N = 16000
K_LDPC = KB * Z          # 8448
NBPS = 4
NQ = N // NBPS           # 4000
PB_BLOCKS = 19           # only pb[0:7232] survives rate matching

B_TOTAL = 2048
N_CORES = 8
B_CORE = B_TOTAL // N_CORES   # 256
P = 64                        # partitions per core
PACK = 4                      # batch rows packed per uint16 lane (nibbles)
NCHUNK = 4                    # output column chunks of 4000

_CACHE = {}


def _base_entries(rows, cols):
    """Recover (base_row, base_col, shift) triplets from lifted index lists."""
    rows = np.asarray(rows, np.int64)
    cols = np.asarray(cols, np.int64)
    m = (rows % Z) == 0
    br = (rows[m] // Z).astype(int)
    bc = (cols[m] // Z).astype(int)
    sh = (cols[m] % Z).astype(int)
    return list(zip(br.tolist(), bc.tolist(), sh.tolist()))


def _group(entries, n_blocks, drop_bc=()):
    g = [[] for _ in range(n_blocks)]
    for br, bc, s in entries:
        if bc in drop_bc or br >= n_blocks:
            continue
        g[br].append((bc, s))
    return g


def _ilv_copies(chunk):
    """Interleaver copy specs for output chunk (cols [chunk*4000, +4000)):
    (tile, blk0, off, nblk, ln, dst_start_within_chunk).

    c_short = u_bits[768:8000] ++ pa[0:1536] ++ pb[0:7232], and
    out[:, 4j+i] = c_short[i*4000 + j]; chunk c covers j in [c*1000,(c+1)*1000).
    """
    spans = ([("u", b, 0, Z) for b in range(2, 20)] + [("u", 20, 0, 320)]
             + [("pa", b, 0, Z) for b in range(4)]
             + [("pb", b, 0, Z) for b in range(18)] + [("pb", 18, 0, 320)])
    jlo, jhi = chunk * (NQ // NCHUNK), (chunk + 1) * (NQ // NCHUNK)
    out = []
    for i in range(NBPS):
        # phase i reads c_short[i*NQ + j] for j in [jlo, jhi) of this chunk
        glo, ghi = i * NQ + jlo, i * NQ + jhi
        g = 0
        pieces = []
        for tname, blk, off, ln in spans:
            a, b = max(g, glo), min(g + ln, ghi)
            if a < b:
                pieces.append((tname, blk, off + a - g, b - a,
                               4 * (a - glo) + i))
            g += ln
        merged = []
        for pc in pieces:
            tname, blk, off, ln, ds = pc
            if merged and off == 0 and ln == Z:
                mt, mb_, mo, mn, ml, mds = merged[-1]
                if mt == tname and mo == 0 and ml == Z and mb_ + mn == blk:
                    merged[-1] = (mt, mb_, mo, mn + 1, ml, mds)
                    continue
            merged.append((tname, blk, off, 1, ln, ds))
        out.extend(merged)
    return out


def _build_program(gA, gC1, gC2):
    import concourse.tile as tile
    from concourse import bacc, mybir
    from concourse.alu_op_type import AluOpType
    import bass_rust

    u16 = mybir.dt.uint16
    XOR = AluOpType.bitwise_xor
    VecI64Pair = bass_rust.VecI64Pair

    nc = bacc.Bacc("TRN2", target_bir_lowering=False, debug=False)
    u_dram = nc.dram_tensor("u", [P, K], u16, kind="ExternalInput").ap()
    o_dram = nc.dram_tensor("out", [P, N], u16, kind="ExternalOutput").ap()

    def pair_view(flat_ap, addr_a, addr_b, ln=Z):
        """[P, 2, ln] view of a flat [P, M] tile AP at two free offsets."""
        v = flat_ap[:, addr_a:addr_a + 1]
        w = v.copy()
        pstride = v.ap.to_list()[0]
        w.ap = VecI64Pair([pstride, [addr_b - addr_a, 2], [1, ln]])
        return w

    with tile.TileContext(nc) as tc, ExitStack() as ctx:
        pin = ctx.enter_context(tc.tile_pool(name="pin", bufs=1))
        pw = ctx.enter_context(tc.tile_pool(name="pw", bufs=1))
        pout = ctx.enter_context(tc.tile_pool(name="pout", bufs=1))

        # u_dup[p, bc*768 + 0:384] = info block bc; [.. 384:768] = halo copy
        # (blocks 0..20; block 21 is all-filler and dropped from the graph)
        u_dup = pin.tile([P, 21 * 2 * Z], u16, tag="u_dup")
        u3 = u_dup.rearrange("p (a b) -> p a b", b=2 * Z)
        nc.gpsimd.memset(u3[:, 20, 320:Z], 0)
        nc.gpsimd.memset(u3[:, 20, Z + 320:2 * Z], 0)
        nc.sync.dma_start(
            u3[:, 0:20, 0:Z],
            u_dram[:, 0:7680].rearrange("p (a b) -> p a b", b=Z))
        nc.sync.dma_start(u3[:, 20, 0:320], u_dram[:, 7680:8000])
        nc.sync.dma_start(
            u3[:, 0:20, Z:2 * Z],
            u_dram[:, 0:7680].rearrange("p (a b) -> p a b", b=Z))
        nc.sync.dma_start(u3[:, 20, Z:Z + 320], u_dram[:, 7680:8000])

        # work tile: au rows 0..3 then pb rows 0..18, each Z wide (flat)
        work = pw.tile([P, (4 + PB_BLOCKS) * Z], u16, tag="work")
        pa = pw.tile([P, 4 * 2 * Z], u16, tag="pa")
        pa3 = pa.rearrange("p (a b) -> p a b", b=2 * Z)

        def uaddr(bc, s):
            return bc * 2 * Z + s

        def paaddr(bc, s):
            return bc * 2 * Z + s

        def au_a(br):
            return br * Z

        def pb_a(r):
            return (4 + r) * Z

        # ---------- wave scheduler: pair-fuse independent chain steps ------
        # rows: au rows then pb rows.  Each row = (dst_addr, [steps]);
        # step = ('first', a1, a2) | ('accu', a) | ('accpa', a).
        emitted = {"n": 0}

        def emit_steps(s1, s2):
            """Emit one DVE instruction covering one or two chain steps."""
            if s2 is None:
                dst_a, st = s1
                if st[0] == "first":
                    nc.vector.tensor_tensor(
                        work[:, dst_a:dst_a + Z],
                        u_dup[:, st[1]:st[1] + Z],
                        u_dup[:, st[2]:st[2] + Z], op=XOR)
                else:
                    src = u_dup if st[0] == "accu" else pa
                    nc.vector.tensor_tensor(
                        work[:, dst_a:dst_a + Z],
                        work[:, dst_a:dst_a + Z],
                        src[:, st[1]:st[1] + Z], op=XOR)
                return
            (da, sta), (db, stb) = s1, s2
            dst = pair_view(work, da, db)
            if sta[0] == "first":
                nc.vector.tensor_tensor(
                    dst, pair_view(u_dup, sta[1], stb[1]),
                    pair_view(u_dup, sta[2], stb[2]), op=XOR)
            else:
                src = u_dup if sta[0] == "accu" else pa
                nc.vector.tensor_tensor(
                    dst, pair_view(work, da, db),
                    pair_view(src, sta[1], stb[1]), op=XOR)

        def run_waves(rows, on_row_done=None):
            """rows: list of (row_key, dst_addr, steps).  Two-server greedy:
            each wave advances the two foremost unfinished rows, fusing
            same-class steps.  Completion order == list order."""
            pos = [0] * len(rows)
            ndone = 0
            while ndone < len(rows):
                # find first two unfinished rows
                act = [i for i in range(len(rows)) if pos[i] < len(rows[i][2])]
                if not act:
                    break
                i = act[0]
                j = act[1] if len(act) > 1 else None
                s1 = (rows[i][1], rows[i][2][pos[i]])
                s2 = None
                if j is not None:
                    stj = rows[j][2][pos[j]]
                    if stj[0] == rows[i][2][pos[i]][0]:
                        s2 = (rows[j][1], stj)
                emit_steps(s1, s2)
                pos[i] += 1
                if s2 is not None:
                    pos[j] += 1
                for k in (i, j):
                    if k is not None and pos[k] == len(rows[k][2]):
                        ndone += 1
                        if on_row_done:
                            on_row_done(rows[k][0])

        def mk_steps(c1_terms, c2_terms):
            steps = []
            ts = [("u", uaddr(bc, s)) for bc, s in c1_terms]
            ts += [("pa", paaddr(bc, s)) for bc, s in c2_terms]
            assert ts[0][0] == "u" and ts[1][0] == "u"
            steps.append(("first", ts[0][1], ts[1][1]))
            for kind, a in ts[2:]:
                steps.append(("accu" if kind == "u" else "accpa", a))
            return steps

        # ---- au = A @ u ----
        au_rows = [(("au", br), au_a(br), mk_steps(gA[br], []))
                   for br in range(4)]
        run_waves(au_rows)

        # ---- pa = B_inv @ au = cumulative XOR; halo for C2 shifts ----
        nc.vector.tensor_copy(pa3[:, 0, 0:Z], work[:, au_a(0):au_a(0) + Z])
        for i in range(1, 4):
            nc.vector.tensor_tensor(pa3[:, i, 0:Z], pa3[:, i - 1, 0:Z],
                                    work[:, au_a(i):au_a(i) + Z], op=XOR)
        nc.scalar.copy(pa3[:, :, Z:2 * Z], pa3[:, :, 0:Z])

        # ---- interleave infrastructure ----
        of = pout.tile([P, N], u16, tag="of")
        cw = N // NCHUNK

        def emit_ilv(c, tname_sel):
            base = c * cw
            for tname, blk0, off, nblk, ln, ds in _ilv_copies(c):
                if tname != tname_sel:
                    continue
                if tname == "u":
                    src_t, bw = u_dup, 2 * Z
                elif tname == "pa":
                    src_t, bw = pa, 2 * Z
                else:
                    src_t, bw = work, Z
                a0 = (blk0 + (4 if tname == "pb" else 0)) * bw + off
                dsa = base + ds
                if nblk > 1:
                    dst = of[:, dsa:dsa + 4 * (Z * nblk - 1) + 1:4]
                    dst = dst.rearrange("p (a b) -> p a b", b=Z)
                    src = src_t[:, a0:a0 + (nblk - 1) * bw + Z]
                    src = src.rearrange("p (a b) -> p a b", b=bw) \
                        if bw == Z else \
                        src_t.rearrange("p (a b) -> p a b", b=bw)[
                            :, blk0:blk0 + nblk, 0:Z]
                    eng = nc.scalar if ln * nblk >= 1000 else nc.gpsimd
                    (eng.copy if eng is nc.scalar else eng.tensor_copy)(
                        dst, src)
                else:
                    dst = of[:, dsa:dsa + 4 * (ln - 1) + 1:4]
                    src = src_t[:, a0:a0 + ln]
                    if ln >= 1000:
                        nc.scalar.copy(dst, src)
                    else:
                        nc.gpsimd.tensor_copy(dst, src)

        # u- and pa-sourced interleave spans only need input DMA / pa chain
        for c in range(NCHUNK):
            emit_ilv(c, "u")
        for c in range(NCHUNK):
            emit_ilv(c, "pa")

        # ---- pb rows, chunk-by-chunk with fused waves ----
        need = []
        for c in range(NCHUNK):
            mx = 0
            for tname, blk0, off, nblk, ln, ds in _ilv_copies(c):
                if tname == "pb":
                    mx = max(mx, blk0 + nblk)
            need.append(mx)

        pb_rows = [(("pb", r), pb_a(r), mk_steps(gC1[r], gC2[r]))
                   for r in range(PB_BLOCKS)]
        done = 0
        for c in range(NCHUNK):
            if need[c] > done:
                run_waves(pb_rows[done:need[c]])
                done = need[c]
            emit_ilv(c, "pb")
            nc.sync.dma_start(o_dram[:, c * cw:(c + 1) * cw],
                              of[:, c * cw:(c + 1) * cw])

    return nc


def _get_program(a_rows, a_cols, bi_rows, bi_cols, c1_rows, c1_cols,
                 c2_rows, c2_cols):
    if "prog" in _CACHE:
        return _CACHE["prog"]
    entB = _base_entries(bi_rows, bi_cols)
    assert sorted(entB) == [(i, j, 0) for i in range(4) for j in range(i + 1)]
    gA = _group(_base_entries(a_rows, a_cols), 4, drop_bc=(21,))
    gC1 = _group(_base_entries(c1_rows, c1_cols), PB_BLOCKS, drop_bc=(21,))
    gC2 = _group(_base_entries(c2_rows, c2_cols), PB_BLOCKS)
    nc = _build_program(gA, gC1, gC2)
    nc.compile()
    _CACHE["prog"] = nc
    return nc


def kernel(u, a_rows, a_cols, bi_rows, bi_cols, c1_rows, c1_cols,
           c2_rows, c2_cols, out_int, **_ignored):
    from concourse.bass_utils import run_bass_kernel_spmd

    u = np.asarray(u)
    assert u.shape == (B_TOTAL, K)
    oi = np.asarray(out_int)
    expect = np.arange(N, dtype=oi.dtype).reshape(NBPS, NQ).T.ravel()
    assert np.array_equal(oi, expect), "unexpected output interleaver"

    nc = _get_program(a_rows, a_cols, bi_rows, bi_cols,
                      c1_rows, c1_cols, c2_rows, c2_cols)

    # host marshalling: pack 4 batch rows per uint16 lane (4-bit nibbles)
    ub = u.astype(np.uint16)
    in_maps = []
    for c in range(N_CORES):
        seg = ub[c * B_CORE:(c + 1) * B_CORE]
        packed = (seg[0 * P:1 * P] | (seg[1 * P:2 * P] << 4)
                  | (seg[2 * P:3 * P] << 8) | (seg[3 * P:4 * P] << 12))
        in_maps.append({"u": np.ascontiguousarray(packed)})

    res = run_bass_kernel_spmd(nc, in_maps, core_ids=list(range(N_CORES)))

    # unpack: nibble t of lane p = batch row t*64+p
    out = np.empty((B_TOTAL, N), np.float32)
    for c in range(N_CORES):
        oc = res.results[c]["out"]
        for t in range(PACK):
            rows = slice(c * B_CORE + t * P, c * B_CORE + (t + 1) * P)
            out[rows] = ((oc >> (4 * t)) & 1).astype(np.float32)
    return out


# revision 7
# speedup vs baseline: 2.5281x; 1.1049x over previous
"""5G LDPC BG1 encoder (k=8000, n=16000, r=0.5, Z=384) on 8 Trainium2 cores.

Strategy: data parallelism over the batch (2048 -> 8 cores x 256 rows) with
4-way nibble packing: 4 batch rows share one uint16 SBUF lane (row t*64+p ->
nibble t of partition p), so every engine op processes 4 codewords at once
and DMA moves 4x fewer bytes.  GF(2) addition is nibble-wise bitwise XOR
(DVE-only on TRN2); nibbles stay in {0,1} so the host recovers bits with a
shift-and-mask.  Circulant shifts use a halo copy of every 384-col block,
loaded by a second DMA pass straight from DRAM.  Independent XOR-chain steps
from two different rows are fused into one [P,2,384] DVE instruction via
hand-built access patterns (arbitrary stride between the two rows), halving
DVE instruction count.  The rate-matching interleaver (out[:,4j+i] =
c_short[i*4000+j]) runs as stride-4 packed copies split between Activation
and GpSimd: u/pa-sourced output phases are emitted early, pb-sourced spans
per chunk as parity rows complete.  Host work is layout-only: pack nibbles
in, shift-and-mask out.
"""
import numpy as np
from contextlib import ExitStack

Z = 384
KB = 22
K = 8000
N = 16000
K_LDPC = KB * Z          # 8448
NBPS = 4
NQ = N // NBPS           # 4000
PB_BLOCKS = 19           # only pb[0:7232] survives rate matching

B_TOTAL = 2048
N_CORES = 8
B_CORE = B_TOTAL // N_CORES   # 256
P = 64                        # partitions per core
PACK = 4                      # batch rows packed per uint16 lane (nibbles)
NCHUNK = 4                    # output column chunks of 4000

_CACHE = {}


def _base_entries(rows, cols):
    """Recover (base_row, base_col, shift) triplets from lifted index lists."""
    rows = np.asarray(rows, np.int64)
    cols = np.asarray(cols, np.int64)
    m = (rows % Z) == 0
    br = (rows[m] // Z).astype(int)
    bc = (cols[m] // Z).astype(int)
    sh = (cols[m] % Z).astype(int)
    return list(zip(br.tolist(), bc.tolist(), sh.tolist()))


def _group(entries, n_blocks, drop_bc=()):
    g = [[] for _ in range(n_blocks)]
    for br, bc, s in entries:
        if bc in drop_bc or br >= n_blocks:
            continue
        g[br].append((bc, s))
    return g


def _ilv_copies(chunk):
    """Interleaver copy specs for output chunk (cols [chunk*4000, +4000)):
    (tile, blk0, off, nblk, ln, dst_start_within_chunk).

    c_short = u_bits[768:8000] ++ pa[0:1536] ++ pb[0:7232], and
    out[:, 4j+i] = c_short[i*4000 + j]; chunk c covers j in [c*1000,(c+1)*1000).
    """
    spans = ([("u", b, 0, Z) for b in range(2, 20)] + [("u", 20, 0, 320)]
             + [("pa", b, 0, Z) for b in range(4)]
             + [("pb", b, 0, Z) for b in range(18)] + [("pb", 18, 0, 320)])
    jlo, jhi = chunk * (NQ // NCHUNK), (chunk + 1) * (NQ // NCHUNK)
    out = []
    for i in range(NBPS):
        # phase i reads c_short[i*NQ + j] for j in [jlo, jhi) of this chunk
        glo, ghi = i * NQ + jlo, i * NQ + jhi
        g = 0
        pieces = []
        for tname, blk, off, ln in spans:
            a, b = max(g, glo), min(g + ln, ghi)
            if a < b:
                pieces.append((tname, blk, off + a - g, b - a,
                               4 * (a - glo) + i))
            g += ln
        merged = []
        for pc in pieces:
            tname, blk, off, ln, ds = pc
            if merged and off == 0 and ln == Z:
                mt, mb_, mo, mn, ml, mds = merged[-1]
                if mt == tname and mo == 0 and ml == Z and mb_ + mn == blk:
                    merged[-1] = (mt, mb_, mo, mn + 1, ml, mds)
                    continue
            merged.append((tname, blk, off, 1, ln, ds))
        out.extend(merged)
    return out


def _build_program(gA, gC1, gC2):
    import concourse.tile as tile
    from concourse import bacc, mybir
    from concourse.alu_op_type import AluOpType
    import bass_rust

    u16 = mybir.dt.uint16
    XOR = AluOpType.bitwise_xor
    VecI64Pair = bass_rust.VecI64Pair

    nc = bacc.Bacc("TRN2", target_bir_lowering=False, debug=False)
    u_dram = nc.dram_tensor("u", [P, K], u16, kind="ExternalInput").ap()
    o_dram = nc.dram_tensor("out", [P, N], u16, kind="ExternalOutput").ap()

    def pair_view(flat_ap, addr_a, addr_b, ln=Z):
        """[P, 2, ln] view of a flat [P, M] tile AP at two free offsets."""
        v = flat_ap[:, addr_a:addr_a + 1]
        w = v.copy()
        pstride = v.ap.to_list()[0]
        w.ap = VecI64Pair([pstride, [addr_b - addr_a, 2], [1, ln]])
        return w

    with tile.TileContext(nc) as tc, ExitStack() as ctx:
        pin = ctx.enter_context(tc.tile_pool(name="pin", bufs=1))
        pw = ctx.enter_context(tc.tile_pool(name="pw", bufs=1))
        pout = ctx.enter_context(tc.tile_pool(name="pout", bufs=1))

        # u_dup[p, bc*768 + 0:384] = info block bc; [.. 384:768] = halo copy
        # (blocks 0..20; block 21 is all-filler and dropped from the graph).
        # DMA in per block-group, main+halo interleaved, so XOR waves that
        # touch low blocks can start while later groups are still in flight.
        u_dup = pin.tile([P, 21 * 2 * Z], u16, tag="u_dup")
        u3 = u_dup.rearrange("p (a b) -> p a b", b=2 * Z)
        nc.gpsimd.memset(u3[:, 20, 320:Z], 0)
        nc.gpsimd.memset(u3[:, 20, Z + 320:2 * Z], 0)
        for lo, hi in ((0, 5), (5, 10), (10, 15), (15, 20)):
            src = u_dram[:, lo * Z:hi * Z].rearrange("p (a b) -> p a b", b=Z)
            nc.sync.dma_start(u3[:, lo:hi, 0:Z], src)
            nc.sync.dma_start(u3[:, lo:hi, Z:2 * Z], src)
        nc.sync.dma_start(u3[:, 20, 0:320], u_dram[:, 7680:8000])
        nc.sync.dma_start(u3[:, 20, Z:Z + 320], u_dram[:, 7680:8000])

        # work tile: au rows 0..3 then pb rows 0..18, each Z wide (flat)
        work = pw.tile([P, (4 + PB_BLOCKS) * Z], u16, tag="work")
        pa = pw.tile([P, 4 * 2 * Z], u16, tag="pa")
        pa3 = pa.rearrange("p (a b) -> p a b", b=2 * Z)

        def uaddr(bc, s):
            return bc * 2 * Z + s

        def paaddr(bc, s):
            return bc * 2 * Z + s

        def au_a(br):
            return br * Z

        def pb_a(r):
            return (4 + r) * Z

        # ---------- wave scheduler: pair-fuse independent chain steps ------
        # rows: au rows then pb rows.  Each row = (dst_addr, [steps]);
        # step = ('first', a1, a2) | ('accu', a) | ('accpa', a).
        emitted = {"n": 0}

        def emit_steps(s1, s2):
            """Emit one DVE instruction covering one or two chain steps."""
            if s2 is None:
                dst_a, st = s1
                if st[0] == "first":
                    nc.vector.tensor_tensor(
                        work[:, dst_a:dst_a + Z],
                        u_dup[:, st[1]:st[1] + Z],
                        u_dup[:, st[2]:st[2] + Z], op=XOR)
                else:
                    src = u_dup if st[0] == "accu" else pa
                    nc.vector.tensor_tensor(
                        work[:, dst_a:dst_a + Z],
                        work[:, dst_a:dst_a + Z],
                        src[:, st[1]:st[1] + Z], op=XOR)
                return
            (da, sta), (db, stb) = s1, s2
            dst = pair_view(work, da, db)
            if sta[0] == "first":
                nc.vector.tensor_tensor(
                    dst, pair_view(u_dup, sta[1], stb[1]),
                    pair_view(u_dup, sta[2], stb[2]), op=XOR)
            else:
                src = u_dup if sta[0] == "accu" else pa
                nc.vector.tensor_tensor(
                    dst, pair_view(work, da, db),
                    pair_view(src, sta[1], stb[1]), op=XOR)

        def run_waves(rows):
            """rows: list of (row_key, dst_addr, state) with state =
            {'first': (a1,a2)|None, 'accu': [a..], 'accpa': [a..]}.
            Two-server greedy by list order: each wave advances the foremost
            unfinished row, fused with the next row that can offer a
            same-class step ('first' must precede accs within a row; accu
            and accpa commute).  Completion order == list order."""
            def avail(st):
                if st["first"] is not None:
                    return ("first",)
                cl = []
                if st["accu"]:
                    cl.append("accu")
                if st["accpa"]:
                    cl.append("accpa")
                return tuple(cl)

            def take(st, cls):
                if cls == "first":
                    a1, a2 = st["first"]
                    st["first"] = None
                    return ("first", a1, a2)
                if cls == "accu":
                    return ("accu", st["accu"].pop(0))
                return ("accpa", st["accpa"].pop(0))

            while True:
                act = [i for i, r in enumerate(rows) if avail(r[2])]
                if not act:
                    break
                i = act[0]
                ci = avail(rows[i][2])
                pick = None
                for j in act[1:]:
                    shared = [c for c in ci if c in avail(rows[j][2])]
                    if shared:
                        pick = (j, shared[0])
                        break
                if pick is None:
                    # prefer draining accu first (pa may not be ready early)
                    cls = ci[0]
                    emit_steps((rows[i][1], take(rows[i][2], cls)), None)
                else:
                    j, cls = pick
                    emit_steps((rows[i][1], take(rows[i][2], cls)),
                               (rows[j][1], take(rows[j][2], cls)))

        def mk_state(c1_terms, c2_terms):
            us = sorted(uaddr(bc, s) for bc, s in c1_terms)
            pas = sorted(paaddr(bc, s) for bc, s in c2_terms)
            assert len(us) >= 2
            return {"first": (us[0], us[1]), "accu": us[2:], "accpa": pas}

        # ---- au = A @ u ----
        au_rows = [(("au", br), au_a(br), mk_state(gA[br], []))
                   for br in range(4)]
        run_waves(au_rows)

        # ---- pa = B_inv @ au = cumulative XOR; halo for C2 shifts ----
        nc.vector.tensor_copy(pa3[:, 0, 0:Z], work[:, au_a(0):au_a(0) + Z])
        for i in range(1, 4):
            nc.vector.tensor_tensor(pa3[:, i, 0:Z], pa3[:, i - 1, 0:Z],
                                    work[:, au_a(i):au_a(i) + Z], op=XOR)

        # ---- interleave infrastructure: balance Act vs Pool by cost ----
        of = pout.tile([P, N], u16, tag="of")
        cw = N // NCHUNK
        ebusy = {"act": 0.0, "pool": 0.0}

        def ilv_op(dst, src, ln):
            ca = 0.833 * ln + 370          # Activation: fast, big fixed cost
            cp = 1.389 * ln + 60           # GpSimd: slower, no fixed cost
            if ebusy["act"] + ca <= ebusy["pool"] + cp:
                ebusy["act"] += ca
                nc.scalar.copy(dst, src)
            else:
                ebusy["pool"] += cp
                nc.gpsimd.tensor_copy(dst, src)

        def emit_ilv(c, tname_sel):
            base = c * cw
            for tname, blk0, off, nblk, ln, ds in _ilv_copies(c):
                if tname != tname_sel:
                    continue
                if tname == "u":
                    src_t, bw = u_dup, 2 * Z
                elif tname == "pa":
                    src_t, bw = pa, 2 * Z
                else:
                    src_t, bw = work, Z
                a0 = (blk0 + (4 if tname == "pb" else 0)) * bw + off
                dsa = base + ds
                if nblk > 1:
                    dst = of[:, dsa:dsa + 4 * (Z * nblk - 1) + 1:4]
                    dst = dst.rearrange("p (a b) -> p a b", b=Z)
                    if bw == Z:
                        src = src_t[:, a0:a0 + (nblk - 1) * bw + Z]
                        src = src.rearrange("p (a b) -> p a b", b=bw)
                    else:
                        src = src_t.rearrange("p (a b) -> p a b", b=bw)[
                            :, blk0:blk0 + nblk, 0:Z]
                    ilv_op(dst, src, ln * nblk)
                else:
                    dst = of[:, dsa:dsa + 4 * (ln - 1) + 1:4]
                    src = src_t[:, a0:a0 + ln]
                    ilv_op(dst, src, ln)

        # u-sourced interleave spans only need the input DMA.  Emit chunk 0
        # first, then the pa halo (ready once the pa chain lands) so C2
        # steps are not blocked behind bulk copies, then the rest.
        emit_ilv(0, "u")
        nc.scalar.copy(pa3[:, :, Z:2 * Z], pa3[:, :, 0:Z])
        for c in range(1, NCHUNK):
            emit_ilv(c, "u")
        for c in range(NCHUNK):
            emit_ilv(c, "pa")

        # ---- pb rows, chunk-by-chunk with fused waves ----
        need = []
        for c in range(NCHUNK):
            mx = 0
            for tname, blk0, off, nblk, ln, ds in _ilv_copies(c):
                if tname == "pb":
                    mx = max(mx, blk0 + nblk)
            need.append(mx)

        pb_rows = [(("pb", r), pb_a(r), mk_state(gC1[r], gC2[r]))
                   for r in range(PB_BLOCKS)]
        done = 0
        for c in range(NCHUNK):
            if need[c] > done:
                run_waves(pb_rows[done:need[c]])
                done = need[c]
            emit_ilv(c, "pb")
            nc.sync.dma_start(o_dram[:, c * cw:(c + 1) * cw],
                              of[:, c * cw:(c + 1) * cw])

    return nc


def _get_program(a_rows, a_cols, bi_rows, bi_cols, c1_rows, c1_cols,
                 c2_rows, c2_cols):
    if "prog" in _CACHE:
        return _CACHE["prog"]
    entB = _base_entries(bi_rows, bi_cols)
    assert sorted(entB) == [(i, j, 0) for i in range(4) for j in range(i + 1)]
    gA = _group(_base_entries(a_rows, a_cols), 4, drop_bc=(21,))
    gC1 = _group(_base_entries(c1_rows, c1_cols), PB_BLOCKS, drop_bc=(21,))
    gC2 = _group(_base_entries(c2_rows, c2_cols), PB_BLOCKS)
    nc = _build_program(gA, gC1, gC2)
    nc.compile()
    _CACHE["prog"] = nc
    return nc


def kernel(u, a_rows, a_cols, bi_rows, bi_cols, c1_rows, c1_cols,
           c2_rows, c2_cols, out_int, **_ignored):
    from concourse.bass_utils import run_bass_kernel_spmd

    u = np.asarray(u)
    assert u.shape == (B_TOTAL, K)
    oi = np.asarray(out_int)
    expect = np.arange(N, dtype=oi.dtype).reshape(NBPS, NQ).T.ravel()
    assert np.array_equal(oi, expect), "unexpected output interleaver"

    nc = _get_program(a_rows, a_cols, bi_rows, bi_cols,
                      c1_rows, c1_cols, c2_rows, c2_cols)

    # host marshalling: pack 4 batch rows per uint16 lane (4-bit nibbles)
    ub = u.astype(np.uint16)
    in_maps = []
    for c in range(N_CORES):
        seg = ub[c * B_CORE:(c + 1) * B_CORE]
        packed = (seg[0 * P:1 * P] | (seg[1 * P:2 * P] << 4)
                  | (seg[2 * P:3 * P] << 8) | (seg[3 * P:4 * P] << 12))
        in_maps.append({"u": np.ascontiguousarray(packed)})

    res = run_bass_kernel_spmd(nc, in_maps, core_ids=list(range(N_CORES)))

    # unpack: nibble t of lane p = batch row t*64+p
    out = np.empty((B_TOTAL, N), np.float32)
    for c in range(N_CORES):
        oc = res.results[c]["out"]
        for t in range(PACK):
            rows = slice(c * B_CORE + t * P, c * B_CORE + (t + 1) * P)
            out[rows] = ((oc >> (4 * t)) & 1).astype(np.float32)
    return out


# revision 12
# speedup vs baseline: 2.5352x; 1.0028x over previous
"""5G LDPC BG1 encoder (k=8000, n=16000, r=0.5, Z=384) on 8 Trainium2 cores.

Strategy: data parallelism over the batch (2048 -> 8 cores x 256 rows) with
4-way nibble packing: 4 batch rows share one uint16 SBUF lane (row t*64+p ->
nibble t of partition p), so every engine op processes 4 codewords at once
and DMA moves 4x fewer bytes.  GF(2) addition is nibble-wise bitwise XOR
(DVE-only on TRN2); nibbles stay in {0,1} so the host recovers bits with a
shift-and-mask.  Circulant shifts use a halo copy of every 384-col block,
loaded by a second DMA pass straight from DRAM.  Independent XOR-chain steps
from two different rows are fused into one [P,2,384] DVE instruction via
hand-built access patterns (arbitrary stride between the two rows), halving
DVE instruction count.  The rate-matching interleaver (out[:,4j+i] =
c_short[i*4000+j]) runs as stride-4 packed copies split between Activation
and GpSimd: u/pa-sourced output phases are emitted early, pb-sourced spans
per chunk as parity rows complete.  Host work is layout-only: pack nibbles
in, shift-and-mask out.
"""
import numpy as np
from contextlib import ExitStack

Z = 384
KB = 22
K = 8000
N = 16000
K_LDPC = KB * Z          # 8448
NBPS = 4
NQ = N // NBPS           # 4000
PB_BLOCKS = 19           # only pb[0:7232] survives rate matching

B_TOTAL = 2048
N_CORES = 8
B_CORE = B_TOTAL // N_CORES   # 256
P = 64                        # partitions per core
PACK = 4                      # batch rows packed per uint16 lane (nibbles)
NCHUNK = 8                    # output column chunks of 2000

_CACHE = {}


def _base_entries(rows, cols):
    """Recover (base_row, base_col, shift) triplets from lifted index lists."""
    rows = np.asarray(rows, np.int64)
    cols = np.asarray(cols, np.int64)
    m = (rows % Z) == 0
    br = (rows[m] // Z).astype(int)
    bc = (cols[m] // Z).astype(int)
    sh = (cols[m] % Z).astype(int)
    return list(zip(br.tolist(), bc.tolist(), sh.tolist()))


def _group(entries, n_blocks, drop_bc=()):
    g = [[] for _ in range(n_blocks)]
    for br, bc, s in entries:
        if bc in drop_bc or br >= n_blocks:
            continue
        g[br].append((bc, s))
    return g


def _ilv_copies(chunk):
    """Interleaver copy specs for output chunk (cols [chunk*4000, +4000)):
    (tile, blk0, off, nblk, ln, dst_start_within_chunk).

    c_short = u_bits[768:8000] ++ pa[0:1536] ++ pb[0:7232], and
    out[:, 4j+i] = c_short[i*4000 + j]; chunk c covers j in [c*1000,(c+1)*1000).
    """
    spans = ([("u", b, 0, Z) for b in range(2, 20)] + [("u", 20, 0, 320)]
             + [("pa", b, 0, Z) for b in range(4)]
             + [("pb", b, 0, Z) for b in range(18)] + [("pb", 18, 0, 320)])
    jlo, jhi = chunk * (NQ // NCHUNK), (chunk + 1) * (NQ // NCHUNK)
    out = []
    for i in range(NBPS):
        # phase i reads c_short[i*NQ + j] for j in [jlo, jhi) of this chunk
        glo, ghi = i * NQ + jlo, i * NQ + jhi
        g = 0
        pieces = []
        for tname, blk, off, ln in spans:
            a, b = max(g, glo), min(g + ln, ghi)
            if a < b:
                pieces.append((tname, blk, off + a - g, b - a,
                               4 * (a - glo) + i))
            g += ln
        merged = []
        for pc in pieces:
            tname, blk, off, ln, ds = pc
            if merged and off == 0 and ln == Z:
                mt, mb_, mo, mn, ml, mds = merged[-1]
                if mt == tname and mo == 0 and ml == Z and mb_ + mn == blk:
                    merged[-1] = (mt, mb_, mo, mn + 1, ml, mds)
                    continue
            merged.append((tname, blk, off, 1, ln, ds))
        out.extend(merged)
    return out


def _build_program(gA, gC1, gC2):
    import concourse.tile as tile
    from concourse import bacc, mybir
    from concourse.alu_op_type import AluOpType
    import bass_rust

    u16 = mybir.dt.uint16
    XOR = AluOpType.bitwise_xor
    VecI64Pair = bass_rust.VecI64Pair

    nc = bacc.Bacc("TRN2", target_bir_lowering=False, debug=False)
    u_dram = nc.dram_tensor("u", [P, K], u16, kind="ExternalInput").ap()
    o_dram = nc.dram_tensor("out", [P, N], u16, kind="ExternalOutput").ap()

    def pair_view(flat_ap, addr_a, addr_b, ln=Z):
        """[P, 2, ln] view of a flat [P, M] tile AP at two free offsets."""
        v = flat_ap[:, addr_a:addr_a + 1]
        w = v.copy()
        pstride = v.ap.to_list()[0]
        w.ap = VecI64Pair([pstride, [addr_b - addr_a, 2], [1, ln]])
        return w

    with tile.TileContext(nc) as tc, ExitStack() as ctx:
        pin = ctx.enter_context(tc.tile_pool(name="pin", bufs=1))
        pw = ctx.enter_context(tc.tile_pool(name="pw", bufs=1))
        pout = ctx.enter_context(tc.tile_pool(name="pout", bufs=1))

        # u_dup[p, bc*768 + 0:384] = info block bc; [.. 384:768] = halo copy
        # (blocks 0..20; block 21 is all-filler and dropped from the graph).
        # DMA in per block-group, main+halo interleaved, so XOR waves that
        # touch low blocks can start while later groups are still in flight.
        u_dup = pin.tile([P, 21 * 2 * Z], u16, tag="u_dup")
        u3 = u_dup.rearrange("p (a b) -> p a b", b=2 * Z)
        nc.gpsimd.memset(u3[:, 20, 320:Z], 0)
        nc.gpsimd.memset(u3[:, 20, Z + 320:2 * Z], 0)
        for lo, hi in ((0, 4), (4, 10), (10, 15), (15, 20)):
            src = u_dram[:, lo * Z:hi * Z].rearrange("p (a b) -> p a b", b=Z)
            nc.sync.dma_start(u3[:, lo:hi, 0:Z], src)
            nc.sync.dma_start(u3[:, lo:hi, Z:2 * Z], src)
        nc.sync.dma_start(u3[:, 20, 0:320], u_dram[:, 7680:8000])
        nc.sync.dma_start(u3[:, 20, Z:Z + 320], u_dram[:, 7680:8000])

        # work tile: au rows 0..3 then pb rows 0..18, each Z wide (flat)
        work = pw.tile([P, (4 + PB_BLOCKS) * Z], u16, tag="work")
        pa = pw.tile([P, 4 * 2 * Z], u16, tag="pa")
        pa3 = pa.rearrange("p (a b) -> p a b", b=2 * Z)

        def uaddr(bc, s):
            return bc * 2 * Z + s

        def paaddr(bc, s):
            return bc * 2 * Z + s

        def au_a(br):
            return br * Z

        def pb_a(r):
            return (4 + r) * Z

        # ---------- wave scheduler: pair-fuse independent chain steps ------
        # rows: au rows then pb rows.  Each row = (dst_addr, [steps]);
        # step = ('first', a1, a2) | ('accu', a) | ('accpa', a).
        emitted = {"n": 0}

        def emit_steps(s1, s2):
            """Emit one DVE instruction covering one or two chain steps."""
            if s2 is None:
                dst_a, st = s1
                if st[0] == "first":
                    nc.vector.tensor_tensor(
                        work[:, dst_a:dst_a + Z],
                        u_dup[:, st[1]:st[1] + Z],
                        u_dup[:, st[2]:st[2] + Z], op=XOR)
                else:
                    src = u_dup if st[0] == "accu" else pa
                    nc.vector.tensor_tensor(
                        work[:, dst_a:dst_a + Z],
                        work[:, dst_a:dst_a + Z],
                        src[:, st[1]:st[1] + Z], op=XOR)
                return
            (da, sta), (db, stb) = s1, s2
            dst = pair_view(work, da, db)
            if sta[0] == "first":
                nc.vector.tensor_tensor(
                    dst, pair_view(u_dup, sta[1], stb[1]),
                    pair_view(u_dup, sta[2], stb[2]), op=XOR)
            else:
                src = u_dup if sta[0] == "accu" else pa
                nc.vector.tensor_tensor(
                    dst, pair_view(work, da, db),
                    pair_view(src, sta[1], stb[1]), op=XOR)

        def run_waves(rows, on_row_done=None):
            """rows: list of (row_key, dst_addr, state) with state =
            {'first': (a1,a2)|None, 'accu': [a..], 'accpa': [a..]}.
            Two-server greedy by list order: each wave advances the foremost
            unfinished row, fused with the next row that can offer a
            same-class step ('first' must precede accs within a row; accu
            and accpa commute).  Completion order == list order."""
            def avail(st):
                if st["first"] is not None:
                    return ("first",)
                cl = []
                if st["accu"]:
                    cl.append("accu")
                if st["accpa"]:
                    cl.append("accpa")
                return tuple(cl)

            def take(st, cls):
                if cls == "first":
                    a1, a2 = st["first"]
                    st["first"] = None
                    return ("first", a1, a2)
                if cls == "accu":
                    return ("accu", st["accu"].pop(0))
                return ("accpa", st["accpa"].pop(0))

            def row_done(k):
                if not avail(rows[k][2]) and on_row_done:
                    on_row_done(rows[k][0])

            while True:
                act = [i for i, r in enumerate(rows) if avail(r[2])]
                if not act:
                    break
                i = act[0]
                ci = avail(rows[i][2])
                pick = None
                for j in act[1:]:
                    shared = [c for c in ci if c in avail(rows[j][2])]
                    if shared:
                        pick = (j, shared[0])
                        break
                if pick is None:
                    # prefer draining accu first (pa may not be ready early)
                    cls = ci[0]
                    emit_steps((rows[i][1], take(rows[i][2], cls)), None)
                    row_done(i)
                else:
                    j, cls = pick
                    emit_steps((rows[i][1], take(rows[i][2], cls)),
                               (rows[j][1], take(rows[j][2], cls)))
                    row_done(i)
                    row_done(j)

        def mk_state(c1_terms, c2_terms):
            us = sorted(uaddr(bc, s) for bc, s in c1_terms)
            pas = sorted(paaddr(bc, s) for bc, s in c2_terms)
            assert len(us) >= 2
            return {"first": (us[0], us[1]), "accu": us[2:], "accpa": pas}

        # ---- au = A @ u ----
        au_rows = [(("au", br), au_a(br), mk_state(gA[br], []))
                   for br in range(4)]
        run_waves(au_rows)

        # ---- pa = B_inv @ au = cumulative XOR, written to main AND halo
        # half in one dual-write op each (no separate halo copy) ----
        nc.vector.tensor_copy(pair_view(pa, 0, Z),
                              pair_view(work, au_a(0), au_a(0)))
        for i in range(1, 4):
            nc.vector.tensor_tensor(
                pair_view(pa, i * 2 * Z, i * 2 * Z + Z),
                pair_view(pa, (i - 1) * 2 * Z, (i - 1) * 2 * Z),
                pair_view(work, au_a(i), au_a(i)), op=XOR)

        # ---- interleave infrastructure: balance Act vs Pool by cost ----
        of = pout.tile([P, N], u16, tag="of")
        cw = N // NCHUNK
        ebusy = {"act": 0.0, "pool": 0.0}

        def ilv_op(dst, src, ln):
            ca = 0.833 * ln + 370          # Activation: fast, big fixed cost
            cp = 1.389 * ln + 60           # GpSimd: slower, no fixed cost
            if ebusy["act"] + ca <= ebusy["pool"] + cp:
                ebusy["act"] += ca
                nc.scalar.copy(dst, src)
            else:
                ebusy["pool"] += cp
                nc.gpsimd.tensor_copy(dst, src)

        def emit_ilv(c, tname_sel):
            base = c * cw
            for tname, blk0, off, nblk, ln, ds in _ilv_copies(c):
                if tname != tname_sel:
                    continue
                if tname == "u":
                    src_t, bw = u_dup, 2 * Z
                elif tname == "pa":
                    src_t, bw = pa, 2 * Z
                else:
                    src_t, bw = work, Z
                a0 = (blk0 + (4 if tname == "pb" else 0)) * bw + off
                dsa = base + ds
                if nblk > 1:
                    dst = of[:, dsa:dsa + 4 * (Z * nblk - 1) + 1:4]
                    dst = dst.rearrange("p (a b) -> p a b", b=Z)
                    if bw == Z:
                        src = src_t[:, a0:a0 + (nblk - 1) * bw + Z]
                        src = src.rearrange("p (a b) -> p a b", b=bw)
                    else:
                        src = src_t.rearrange("p (a b) -> p a b", b=bw)[
                            :, blk0:blk0 + nblk, 0:Z]
                    ilv_op(dst, src, ln * nblk)
                else:
                    dst = of[:, dsa:dsa + 4 * (ln - 1) + 1:4]
                    src = src_t[:, a0:a0 + ln]
                    ilv_op(dst, src, ln)

        # u- and pa-sourced interleave spans only need input DMA / pa chain
        for c in range(NCHUNK):
            emit_ilv(c, "u")
        for c in range(NCHUNK):
            emit_ilv(c, "pa")

        # ---- pb rows: one global wave pass (max pair-fusion), rows ordered
        # by first-needing chunk; emit each chunk's pb spans + DMA as soon
        # as every row it needs has completed ----
        needset = []
        for c in range(NCHUNK):
            s = set()
            for tname, blk0, off, nblk, ln, ds in _ilv_copies(c):
                if tname == "pb":
                    s.update(range(blk0, blk0 + nblk))
            needset.append(s)

        prio = []
        for c in range(NCHUNK):
            for r in sorted(needset[c]):
                if r not in prio:
                    prio.append(r)
        for r in range(PB_BLOCKS):
            if r not in prio:
                prio.append(r)

        rows_done = set()
        next_chunk = [0]

        def flush_chunks(force=False):
            while next_chunk[0] < NCHUNK:
                c = next_chunk[0]
                if not (force or needset[c] <= rows_done):
                    return
                emit_ilv(c, "pb")
                nc.sync.dma_start(o_dram[:, c * cw:(c + 1) * cw],
                                  of[:, c * cw:(c + 1) * cw])
                next_chunk[0] += 1

        def on_done(key):
            rows_done.add(key[1])
            flush_chunks()

        pb_rows = [(("pb", r), pb_a(r), mk_state(gC1[r], gC2[r]))
                   for r in prio]
        run_waves(pb_rows, on_row_done=on_done)
        flush_chunks(force=True)

    return nc


def _get_program(a_rows, a_cols, bi_rows, bi_cols, c1_rows, c1_cols,
                 c2_rows, c2_cols):
    if "prog" in _CACHE:
        return _CACHE["prog"]
    entB = _base_entries(bi_rows, bi_cols)
    assert sorted(entB) == [(i, j, 0) for i in range(4) for j in range(i + 1)]
    gA = _group(_base_entries(a_rows, a_cols), 4, drop_bc=(21,))
    gC1 = _group(_base_entries(c1_rows, c1_cols), PB_BLOCKS, drop_bc=(21,))
    gC2 = _group(_base_entries(c2_rows, c2_cols), PB_BLOCKS)
    nc = _build_program(gA, gC1, gC2)
    nc.compile()
    _CACHE["prog"] = nc
    return nc


def kernel(u, a_rows, a_cols, bi_rows, bi_cols, c1_rows, c1_cols,
           c2_rows, c2_cols, out_int, **_ignored):
    from concourse.bass_utils import run_bass_kernel_spmd

    u = np.asarray(u)
    assert u.shape == (B_TOTAL, K)
    oi = np.asarray(out_int)
    expect = np.arange(N, dtype=oi.dtype).reshape(NBPS, NQ).T.ravel()
    assert np.array_equal(oi, expect), "unexpected output interleaver"

    nc = _get_program(a_rows, a_cols, bi_rows, bi_cols,
                      c1_rows, c1_cols, c2_rows, c2_cols)

    # host marshalling: pack 4 batch rows per uint16 lane (4-bit nibbles)
    ub = u.astype(np.uint16)
    in_maps = []
    for c in range(N_CORES):
        seg = ub[c * B_CORE:(c + 1) * B_CORE]
        packed = (seg[0 * P:1 * P] | (seg[1 * P:2 * P] << 4)
                  | (seg[2 * P:3 * P] << 8) | (seg[3 * P:4 * P] << 12))
        in_maps.append({"u": np.ascontiguousarray(packed)})

    res = run_bass_kernel_spmd(nc, in_maps, core_ids=list(range(N_CORES)))

    # unpack: nibble t of lane p = batch row t*64+p
    out = np.empty((B_TOTAL, N), np.float32)
    for c in range(N_CORES):
        oc = res.results[c]["out"]
        for t in range(PACK):
            rows = slice(c * B_CORE + t * P, c * B_CORE + (t + 1) * P)
            out[rows] = ((oc >> (4 * t)) & 1).astype(np.float32)
    return out


# revision 15
# speedup vs baseline: 2.5402x; 1.0020x over previous
"""5G LDPC BG1 encoder (k=8000, n=16000, r=0.5, Z=384) on 8 Trainium2 cores.

Strategy: data parallelism over the batch (2048 -> 8 cores x 256 rows) with
4-way nibble packing: 4 batch rows share one uint16 SBUF lane (row t*64+p ->
nibble t of partition p), so every engine op processes 4 codewords at once
and DMA moves 4x fewer bytes.  GF(2) addition is nibble-wise bitwise XOR
(DVE-only on TRN2); nibbles stay in {0,1} so the host recovers bits with a
shift-and-mask.  Circulant shifts use a halo copy of every 384-col block,
loaded by a second DMA pass straight from DRAM.  Independent XOR-chain steps
from two different rows are fused into one [P,2,384] DVE instruction via
hand-built access patterns (arbitrary stride between the two rows), halving
DVE instruction count.  The rate-matching interleaver (out[:,4j+i] =
c_short[i*4000+j]) runs as stride-4 packed copies split between Activation
and GpSimd: u/pa-sourced output phases are emitted early, pb-sourced spans
per chunk as parity rows complete.  Host work is layout-only: pack nibbles
in, shift-and-mask out.
"""
import numpy as np
from contextlib import ExitStack

Z = 384
KB = 22
K = 8000
N = 16000
K_LDPC = KB * Z          # 8448
NBPS = 4
NQ = N // NBPS           # 4000
PB_BLOCKS = 19           # only pb[0:7232] survives rate matching

B_TOTAL = 2048
N_CORES = 8
B_CORE = B_TOTAL // N_CORES   # 256
P = 64                        # partitions per core
PACK = 4                      # batch rows packed per uint16 lane (nibbles)
NCHUNK = 8                    # output column chunks of 2000

_CACHE = {}


def _base_entries(rows, cols):
    """Recover (base_row, base_col, shift) triplets from lifted index lists."""
    rows = np.asarray(rows, np.int64)
    cols = np.asarray(cols, np.int64)
    m = (rows % Z) == 0
    br = (rows[m] // Z).astype(int)
    bc = (cols[m] // Z).astype(int)
    sh = (cols[m] % Z).astype(int)
    return list(zip(br.tolist(), bc.tolist(), sh.tolist()))


def _group(entries, n_blocks, drop_bc=()):
    g = [[] for _ in range(n_blocks)]
    for br, bc, s in entries:
        if bc in drop_bc or br >= n_blocks:
            continue
        g[br].append((bc, s))
    return g


def _ilv_copies(chunk, nchunk=NCHUNK):
    """Interleaver copy specs for output chunk (cols [chunk*cw, +cw)):
    (tile, blk0, off, nblk, ln, dst_start_within_chunk).

    c_short = u_bits[768:8000] ++ pa[0:1536] ++ pb[0:7232], and
    out[:, 4j+i] = c_short[i*4000 + j]; chunk c covers j in
    [c*(NQ/nchunk), (c+1)*(NQ/nchunk)).
    """
    spans = ([("u", b, 0, Z) for b in range(2, 20)] + [("u", 20, 0, 320)]
             + [("pa", b, 0, Z) for b in range(4)]
             + [("pb", b, 0, Z) for b in range(18)] + [("pb", 18, 0, 320)])
    jlo, jhi = chunk * (NQ // nchunk), (chunk + 1) * (NQ // nchunk)
    out = []
    for i in range(NBPS):
        # phase i reads c_short[i*NQ + j] for j in [jlo, jhi); dst offsets
        # are absolute within the [P, N] output tile.
        glo, ghi = i * NQ + jlo, i * NQ + jhi
        g = 0
        pieces = []
        for tname, blk, off, ln in spans:
            a, b = max(g, glo), min(g + ln, ghi)
            if a < b:
                pieces.append((tname, blk, off + a - g, b - a,
                               4 * (a - glo) + i + 4 * jlo))
            g += ln
        merged = []
        for pc in pieces:
            tname, blk, off, ln, ds = pc
            if merged and off == 0 and ln == Z:
                mt, mb_, mo, mn, ml, mds = merged[-1]
                if mt == tname and mo == 0 and ml == Z and mb_ + mn == blk:
                    merged[-1] = (mt, mb_, mo, mn + 1, ml, mds)
                    continue
            merged.append((tname, blk, off, 1, ln, ds))
        out.extend(merged)
    return out


def _build_program(gA, gC1, gC2):
    import concourse.tile as tile
    from concourse import bacc, mybir
    from concourse.alu_op_type import AluOpType
    import bass_rust

    u16 = mybir.dt.uint16
    XOR = AluOpType.bitwise_xor
    VecI64Pair = bass_rust.VecI64Pair

    nc = bacc.Bacc("TRN2", target_bir_lowering=False, debug=False)
    u_dram = nc.dram_tensor("u", [P, K], u16, kind="ExternalInput").ap()
    o_dram = nc.dram_tensor("out", [P, N], u16, kind="ExternalOutput").ap()

    def pair_view(flat_ap, addr_a, addr_b, ln=Z):
        """[P, 2, ln] view of a flat [P, M] tile AP at two free offsets."""
        v = flat_ap[:, addr_a:addr_a + 1]
        w = v.copy()
        pstride = v.ap.to_list()[0]
        w.ap = VecI64Pair([pstride, [addr_b - addr_a, 2], [1, ln]])
        return w

    with tile.TileContext(nc) as tc, ExitStack() as ctx:
        pin = ctx.enter_context(tc.tile_pool(name="pin", bufs=1))
        pw = ctx.enter_context(tc.tile_pool(name="pw", bufs=1))
        pout = ctx.enter_context(tc.tile_pool(name="pout", bufs=1))

        # u_dup[p, bc*768 + 0:384] = info block bc; [.. 384:768] = halo copy
        # (blocks 0..20; block 21 is all-filler and dropped from the graph).
        # DMA in per block-group, main+halo interleaved, so XOR waves that
        # touch low blocks can start while later groups are still in flight.
        u_dup = pin.tile([P, 21 * 2 * Z], u16, tag="u_dup")
        u3 = u_dup.rearrange("p (a b) -> p a b", b=2 * Z)
        nc.gpsimd.memset(u3[:, 20, 320:Z], 0)
        nc.gpsimd.memset(u3[:, 20, Z + 320:2 * Z], 0)
        for lo, hi in ((0, 4), (4, 10), (10, 15), (15, 20)):
            src = u_dram[:, lo * Z:hi * Z].rearrange("p (a b) -> p a b", b=Z)
            nc.sync.dma_start(u3[:, lo:hi, 0:Z], src)
            nc.sync.dma_start(u3[:, lo:hi, Z:2 * Z], src)
        nc.sync.dma_start(u3[:, 20, 0:320], u_dram[:, 7680:8000])
        nc.sync.dma_start(u3[:, 20, Z:Z + 320], u_dram[:, 7680:8000])

        # work tile: au rows 0..3 then pb rows 0..18, each Z wide (flat)
        work = pw.tile([P, (4 + PB_BLOCKS) * Z], u16, tag="work")
        pa = pw.tile([P, 4 * 2 * Z], u16, tag="pa")
        pa3 = pa.rearrange("p (a b) -> p a b", b=2 * Z)

        def uaddr(bc, s):
            return bc * 2 * Z + s

        def paaddr(bc, s):
            return bc * 2 * Z + s

        def au_a(br):
            return br * Z

        def pb_a(r):
            return (4 + r) * Z

        # ---------- wave scheduler: pair-fuse independent chain steps ------
        # rows: au rows then pb rows.  Each row = (dst_addr, [steps]);
        # step = ('first', a1, a2) | ('accu', a) | ('accpa', a).
        emitted = {"n": 0}

        def emit_steps(s1, s2):
            """Emit one DVE instruction covering one or two chain steps."""
            if s2 is None:
                dst_a, st = s1
                if st[0] == "first":
                    nc.vector.tensor_tensor(
                        work[:, dst_a:dst_a + Z],
                        u_dup[:, st[1]:st[1] + Z],
                        u_dup[:, st[2]:st[2] + Z], op=XOR)
                else:
                    src = u_dup if st[0] == "accu" else pa
                    nc.vector.tensor_tensor(
                        work[:, dst_a:dst_a + Z],
                        work[:, dst_a:dst_a + Z],
                        src[:, st[1]:st[1] + Z], op=XOR)
                return
            (da, sta), (db, stb) = s1, s2
            dst = pair_view(work, da, db)
            if sta[0] == "first":
                nc.vector.tensor_tensor(
                    dst, pair_view(u_dup, sta[1], stb[1]),
                    pair_view(u_dup, sta[2], stb[2]), op=XOR)
            else:
                src = u_dup if sta[0] == "accu" else pa
                nc.vector.tensor_tensor(
                    dst, pair_view(work, da, db),
                    pair_view(src, sta[1], stb[1]), op=XOR)

        def run_waves(rows, on_row_done=None):
            """rows: list of (row_key, dst_addr, state) with state =
            {'first': (a1,a2)|None, 'accu': [a..], 'accpa': [a..]}.
            Two-server greedy by list order: each wave advances the foremost
            unfinished row, fused with the next row that can offer a
            same-class step ('first' must precede accs within a row; accu
            and accpa commute).  Completion order == list order."""
            def avail(st):
                if st["first"] is not None:
                    return ("first",)
                cl = []
                if st["accu"]:
                    cl.append("accu")
                if st["accpa"]:
                    cl.append("accpa")
                return tuple(cl)

            def take(st, cls):
                if cls == "first":
                    a1, a2 = st["first"]
                    st["first"] = None
                    return ("first", a1, a2)
                if cls == "accu":
                    return ("accu", st["accu"].pop(0))
                return ("accpa", st["accpa"].pop(0))

            def row_done(k):
                if not avail(rows[k][2]) and on_row_done:
                    on_row_done(rows[k][0])

            while True:
                act = [i for i, r in enumerate(rows) if avail(r[2])]
                if not act:
                    break
                i = act[0]
                ci = avail(rows[i][2])
                pick = None
                for j in act[1:]:
                    shared = [c for c in ci if c in avail(rows[j][2])]
                    if shared:
                        pick = (j, shared[0])
                        break
                if pick is None:
                    # prefer draining accu first (pa may not be ready early)
                    cls = ci[0]
                    emit_steps((rows[i][1], take(rows[i][2], cls)), None)
                    row_done(i)
                else:
                    j, cls = pick
                    emit_steps((rows[i][1], take(rows[i][2], cls)),
                               (rows[j][1], take(rows[j][2], cls)))
                    row_done(i)
                    row_done(j)

        def mk_state(c1_terms, c2_terms):
            us = sorted(uaddr(bc, s) for bc, s in c1_terms)
            pas = sorted(paaddr(bc, s) for bc, s in c2_terms)
            assert len(us) >= 2
            return {"first": (us[0], us[1]), "accu": us[2:], "accpa": pas}

        # ---- au = A @ u ----
        au_rows = [(("au", br), au_a(br), mk_state(gA[br], []))
                   for br in range(4)]
        run_waves(au_rows)

        # ---- pa = B_inv @ au = cumulative XOR, written to main AND halo
        # half in one dual-write op each (no separate halo copy) ----
        nc.vector.tensor_copy(pair_view(pa, 0, Z),
                              pair_view(work, au_a(0), au_a(0)))
        for i in range(1, 4):
            nc.vector.tensor_tensor(
                pair_view(pa, i * 2 * Z, i * 2 * Z + Z),
                pair_view(pa, (i - 1) * 2 * Z, (i - 1) * 2 * Z),
                pair_view(work, au_a(i), au_a(i)), op=XOR)

        # ---- pb rows 16..18: integer add-accumulate chains on the software
        # DGE (nibble counts <= 15; host parity-extracts).  Costs ~1us of
        # GpSimd time per hop but runs off the DVE critical path.  WAW on
        # the destination serializes each chain; u-hops first so the queue
        # never stalls waiting for pa. ----
        dma_rows = tuple(r for r in (16, 17, 18)
                         if len(gC1[r]) + len(gC2[r]) <= 15)
        hops_u, hops_pa = [], []
        for r in dma_rows:
            us = sorted(uaddr(bc, s) for bc, s in gC1[r])
            pas = sorted(paaddr(bc, s) for bc, s in gC2[r])
            hops_u.append((r, us))
            hops_pa.append((r, pas))
        for r, us in hops_u:
            dst = work[:, pb_a(r):pb_a(r) + Z]
            nc.gpsimd.dma_start(dst, u_dup[:, us[0]:us[0] + Z])
            for a in us[1:]:
                nc.gpsimd.dma_start(dst, u_dup[:, a:a + Z],
                                    accum_op=AluOpType.add)
        for r, pas in hops_pa:
            dst = work[:, pb_a(r):pb_a(r) + Z]
            for a in pas:
                nc.gpsimd.dma_start(dst, pa[:, a:a + Z],
                                    accum_op=AluOpType.add)

        # ---- interleave: balance Act vs Pool by cost (Pool pre-loaded
        # with the SWDGE hop cost above) ----
        of = pout.tile([P, N], u16, tag="of")
        cw = N // NCHUNK
        nhops = sum(len(us) for _, us in hops_u) + sum(
            len(p) for _, p in hops_pa)
        ebusy = {"act": 0.0, "pool": 1020.0 * nhops}

        def ilv_op(dst, src, ln):
            ca = 0.833 * ln + 370          # Activation: fast, big fixed cost
            cp = 1.389 * ln + 60           # GpSimd: slower, no fixed cost
            if ebusy["act"] + ca <= ebusy["pool"] + cp:
                ebusy["act"] += ca
                nc.scalar.copy(dst, src)
            else:
                ebusy["pool"] += cp
                nc.gpsimd.tensor_copy(dst, src)

        def emit_ilv(c, tname_sel, nchunk=NCHUNK):
            for tname, blk0, off, nblk, ln, ds in _ilv_copies(c, nchunk):
                if tname != tname_sel:
                    continue
                if tname == "u":
                    src_t, bw = u_dup, 2 * Z
                elif tname == "pa":
                    src_t, bw = pa, 2 * Z
                else:
                    src_t, bw = work, Z
                a0 = (blk0 + (4 if tname == "pb" else 0)) * bw + off
                if nblk > 1:
                    dst = of[:, ds:ds + 4 * (Z * nblk - 1) + 1:4]
                    dst = dst.rearrange("p (a b) -> p a b", b=Z)
                    if bw == Z:
                        src = src_t[:, a0:a0 + (nblk - 1) * bw + Z]
                        src = src.rearrange("p (a b) -> p a b", b=bw)
                    else:
                        src = src_t.rearrange("p (a b) -> p a b", b=bw)[
                            :, blk0:blk0 + nblk, 0:Z]
                    ilv_op(dst, src, ln * nblk)
                else:
                    dst = of[:, ds:ds + 4 * (ln - 1) + 1:4]
                    src = src_t[:, a0:a0 + ln]
                    ilv_op(dst, src, ln)

        # u/pa-sourced spans only need input DMA / pa chain; emit at coarse
        # granularity (fewer, bigger copies)
        for c in range(4):
            emit_ilv(c, "u", nchunk=4)
        for c in range(4):
            emit_ilv(c, "pa", nchunk=4)

        # ---- remaining pb rows: one global wave pass (max pair-fusion),
        # rows ordered by first-needing chunk; emit each chunk's pb spans +
        # DMA as soon as every row it needs has completed ----
        needset = []
        for c in range(NCHUNK):
            s = set()
            for tname, blk0, off, nblk, ln, ds in _ilv_copies(c):
                if tname == "pb":
                    s.update(range(blk0, blk0 + nblk))
            needset.append(s)

        prio = []
        for c in range(NCHUNK):
            for r in sorted(needset[c]):
                if r not in prio and r not in dma_rows:
                    prio.append(r)
        for r in range(PB_BLOCKS):
            if r not in prio and r not in dma_rows:
                prio.append(r)

        rows_done = set(dma_rows)
        next_chunk = [0]

        def flush_chunks(force=False):
            while next_chunk[0] < NCHUNK:
                c = next_chunk[0]
                if not (force or needset[c] <= rows_done):
                    return
                emit_ilv(c, "pb")
                nc.sync.dma_start(o_dram[:, c * cw:(c + 1) * cw],
                                  of[:, c * cw:(c + 1) * cw])
                next_chunk[0] += 1

        def on_done(key):
            rows_done.add(key[1])
            flush_chunks()

        pb_rows = [(("pb", r), pb_a(r), mk_state(gC1[r], gC2[r]))
                   for r in prio]
        run_waves(pb_rows, on_row_done=on_done)
        flush_chunks(force=True)

    return nc


def _get_program(a_rows, a_cols, bi_rows, bi_cols, c1_rows, c1_cols,
                 c2_rows, c2_cols):
    if "prog" in _CACHE:
        return _CACHE["prog"]
    entB = _base_entries(bi_rows, bi_cols)
    assert sorted(entB) == [(i, j, 0) for i in range(4) for j in range(i + 1)]
    gA = _group(_base_entries(a_rows, a_cols), 4, drop_bc=(21,))
    gC1 = _group(_base_entries(c1_rows, c1_cols), PB_BLOCKS, drop_bc=(21,))
    gC2 = _group(_base_entries(c2_rows, c2_cols), PB_BLOCKS)
    nc = _build_program(gA, gC1, gC2)
    nc.compile()
    _CACHE["prog"] = nc
    return nc


def kernel(u, a_rows, a_cols, bi_rows, bi_cols, c1_rows, c1_cols,
           c2_rows, c2_cols, out_int, **_ignored):
    from concourse.bass_utils import run_bass_kernel_spmd

    u = np.asarray(u)
    assert u.shape == (B_TOTAL, K)
    oi = np.asarray(out_int)
    expect = np.arange(N, dtype=oi.dtype).reshape(NBPS, NQ).T.ravel()
    assert np.array_equal(oi, expect), "unexpected output interleaver"

    nc = _get_program(a_rows, a_cols, bi_rows, bi_cols,
                      c1_rows, c1_cols, c2_rows, c2_cols)

    # host marshalling: pack 4 batch rows per uint16 lane (4-bit nibbles)
    ub = u.astype(np.uint16)
    in_maps = []
    for c in range(N_CORES):
        seg = ub[c * B_CORE:(c + 1) * B_CORE]
        packed = (seg[0 * P:1 * P] | (seg[1 * P:2 * P] << 4)
                  | (seg[2 * P:3 * P] << 8) | (seg[3 * P:4 * P] << 12))
        in_maps.append({"u": np.ascontiguousarray(packed)})

    res = run_bass_kernel_spmd(nc, in_maps, core_ids=list(range(N_CORES)))

    # unpack: nibble t of lane p = batch row t*64+p
    out = np.empty((B_TOTAL, N), np.float32)
    for c in range(N_CORES):
        oc = res.results[c]["out"]
        for t in range(PACK):
            rows = slice(c * B_CORE + t * P, c * B_CORE + (t + 1) * P)
            out[rows] = ((oc >> (4 * t)) & 1).astype(np.float32)
    return out
